# revision 1
# baseline (speedup 1.0000x reference)
"""CliffordLinear (Cl(3,0)) Trainium2 kernel.

Math: Cl(3,0) is isomorphic to the algebra of 2x2 complex matrices via the
Pauli-matrix representation phi(x) = sum_a x_a * (s1^b0 s2^b1 s3^b2).  The
reference computes out[b,o] = sum_i W[o,i] * X[b,i] (Clifford product per
channel pair), which maps to OutM[b,o] = sum_i phi(W[o,i]) @ phi(X[b,i]) --
a 2x2 complex matrix contraction.  Splitting by output column c and
expanding complex arithmetic into real matmuls gives, per c in {0,1}:

    OutRe_c[b,(o,r)] = XRe_c @ R - XIm_c @ I
    OutIm_c[b,(o,r)] = XRe_c @ I + XIm_c @ R

with R/I = Re/Im of phi(W)[r,m] as [(i,m) x (o,r)] 512x512 matrices.  That
is 17.2G real MACs total vs 34.4G for the naive blade expansion (2x fewer).
The blade <-> Pauli basis changes are 8-point +- butterflies: the input side
is folded into host-side shard prep; the output side runs on the DVE while
evicting PSUM.  Matmuls run in float32r (TF32-like, ~1.5e-4 rel err, full
PE rate; plain fp32 is 4x slower).

Sharding: data-parallel over batch (1024 rows/core); weights replicated.
Per-core HBM traffic: 8.4 MB x + 2.1 MB w in, 8.4 MB out.
"""

import sys

sys.path.insert(0, "/opt/trn_rl_repo")

import numpy as np

import concourse.bass as bass  # noqa: F401  (registers lowerings)
import concourse.mybir as mybir
import concourse.tile as tile
from concourse import bacc
from concourse.bass_utils import run_bass_kernel_spmd

N_CORES = 8
B, CIN, COUT, NB = 8192, 256, 256, 8
BS = B // N_CORES          # 1024 batch rows per core
K = CIN * 4                # 1024 contraction rows (both halves)
HK = K // 2                # 512: rows per Re/Im half
OUTW = COUT * NB           # 2048 output width (o major, blade minor)
KT = K // 128              # 8 k-tiles of the x operand
BT = BS // 128             # 8 b-tiles

_cached = {}


def _build_nc():
    fr = mybir.dt.float32r
    f32 = mybir.dt.float32
    nc = bacc.Bacc("TRN2", target_bir_lowering=False, debug=False,
                   num_devices=N_CORES)
    # x'[c] layout: [bt, p, k, b] so each per-partition row is 4 KiB contiguous
    xt0 = nc.dram_tensor("xt0", [BT, 128, KT * 128], f32, kind="ExternalInput")
    xt1 = nc.dram_tensor("xt1", [BT, 128, KT * 128], f32, kind="ExternalInput")
    # weight planes R|I stacked: [2, 512, 512] = [Re/Im, (i,m), (o,r)]
    wri = nc.dram_tensor("wri", [2, HK, HK], f32, kind="ExternalInput")
    out = nc.dram_tensor("out", [BS, OUTW], f32, kind="ExternalOutput")

    with tile.TileContext(nc) as tc:
        with tc.tile_pool(name="wpool", bufs=1) as wpool, \
             tc.tile_pool(name="xpool", bufs=4) as xpool, \
             tc.tile_pool(name="opool", bufs=3) as opool, \
             tc.tile_pool(name="pspool", bufs=2, space="PSUM") as pspool:
            # PE warmup: ramp the clock gate during the initial DMA wait so
            # real matmuls start at full speed.  Zeros in, result unused.
            warm_in = wpool.tile([128, 640], mybir.dt.bfloat16, tag="warm_in")
            nc.vector.memset(warm_in[:], 0.0)
            warm_ps = pspool.tile([128, 512], f32, tag="ps0")
            for _ in range(8):
                nc.tensor.matmul(warm_ps[:], warm_in[:, :128], warm_in[:, 128:640],
                                 start=True, stop=True)

            # Startup interleave: bt0's x0 arrives in two 256 KiB chunks
            # around the weight-plane DMAs, so the first matmuls begin
            # after ~1.5 us of DMA instead of after the full 3 MB preload.
            x1_pre = xpool.tile([128, KT * 128], fr, tag="x1")
            x0_chunks = []
            for h in range(2):
                x0ph = xpool.tile([128, 512], fr, tag=f"x0p{h}", bufs=1)
                x0_chunks.append(x0ph)
            nc.sync.dma_start(x0_chunks[0][:], xt0[0][:, 0:512].bitcast(fr))
            r_t, i_t, ni_t = [], [], []
            for k in range(4):
                ik = wpool.tile([128, HK], fr, tag=f"i{k}")
                nc.sync.dma_start(ik[:], wri[1, k * 128:(k + 1) * 128, :].bitcast(fr))
                rk = wpool.tile([128, HK], fr, tag=f"r{k}")
                nc.sync.dma_start(rk[:], wri[0, k * 128:(k + 1) * 128, :].bitcast(fr))
                nik = wpool.tile([128, HK], fr, tag=f"ni{k}")
                nc.scalar.mul(nik[:], ik[:].bitcast(f32), -1.0)
                r_t.append(rk); i_t.append(ik); ni_t.append(nik)
            # x0's second chunk is first needed at k=4, after all w-planes
            nc.sync.dma_start(x0_chunks[1][:], xt0[0][:, 512:1024].bitcast(fr))
            nc.sync.dma_start(x1_pre[:], xt1[0].bitcast(fr))
            # rhs per (half, k): Re half: [R0..R3, -I0..-I3]; Im: [I0..I3, R0..R3]
            rhs_re = r_t + ni_t
            rhs_im = i_t + r_t

            for bt in range(BT):
                if bt == 0:
                    x0_lhs = [x0_chunks[k // 4][:, (k % 4) * 128:(k % 4 + 1) * 128]
                              for k in range(KT)]
                    x1_s = x1_pre
                else:
                    x0_s = xpool.tile([128, KT * 128], fr, tag="x0")
                    x1_s = xpool.tile([128, KT * 128], fr, tag="x1")
                    nc.sync.dma_start(x0_s[:], xt0[bt].bitcast(fr))
                    nc.sync.dma_start(x1_s[:], xt1[bt].bitcast(fr))
                    x0_lhs = [x0_s[:, k * 128:(k + 1) * 128] for k in range(KT)]
                x1_lhs = [x1_s[:, k * 128:(k + 1) * 128] for k in range(KT)]
                ps0 = pspool.tile([128, K], f32, tag="ps0")
                ps1 = pspool.tile([128, K], f32, tag="ps1")
                last = bt == BT - 1
                if not last:
                    for xlhs, ps in ((x0_lhs, ps0), (x1_lhs, ps1)):
                        for k in range(KT):
                            # Im first: its rhs never depends on the ScalarE
                            # negation, so a late nI_k can't stall it in the
                            # PE queue.
                            nc.tensor.matmul(ps[:, HK:K], xlhs[k], rhs_im[k][:],
                                             start=(k == 0), stop=(k == KT - 1))
                            nc.tensor.matmul(ps[:, 0:HK], xlhs[k], rhs_re[k][:],
                                             start=(k == 0), stop=(k == KT - 1))
                else:
                    # c1 first (so its eviction overlaps c0), and c0 split in
                    # two column chunks with separate PSUM tiles so chunk A's
                    # butterfly+store overlap chunk B's matmuls.
                    for k in range(KT):
                        nc.tensor.matmul(ps1[:, HK:K], x1_lhs[k], rhs_im[k][:],
                                         start=(k == 0), stop=(k == KT - 1))
                        nc.tensor.matmul(ps1[:, 0:HK], x1_lhs[k], rhs_re[k][:],
                                         start=(k == 0), stop=(k == KT - 1))
                    ps0a = ps0  # reuse the already-allocated ps0 slot: chunk A
                    ps0b = pspool.tile([128, K], f32, tag="ps1")
                    # Re chunk in bank 0, Im chunk in bank 1 (interleaved
                    # accumulation groups must not share a PSUM bank)
                    for cs, pst in ((0, ps0a), (1, ps0b)):
                        for k in range(KT):
                            nc.tensor.matmul(
                                pst[:, 0:256], x0_lhs[k],
                                rhs_re[k][:, cs * 256:(cs + 1) * 256],
                                start=(k == 0), stop=(k == KT - 1))
                            nc.tensor.matmul(
                                pst[:, HK:HK + 256], x0_lhs[k],
                                rhs_im[k][:, cs * 256:(cs + 1) * 256],
                                start=(k == 0), stop=(k == KT - 1))
                stage = opool.tile([128, OUTW], f32, tag="stage")
                # DVE reads only one PSUM operand: evict ps1 via ScalarE
                s1 = opool.tile([128, K], f32, tag="s1")
                nc.scalar.copy(s1[:], ps1[:])
                # inverse Pauli butterfly into blade-minor layout.
                # ps cols: [Re(o,r) | Im(o,r)], (o,r) packed o*2+r.
                # A=P00 (ps0,r0)  C=P10 (ps0,r1)  B=P01 (ps1,r0)  D=P11 (ps1,r1)
                # 4 dual-blade ops via 2-dim free APs (j picks Re/Im half):
                #   add (x0,x7): out 8o+7j      = ps0[512j+2o]   + s1[512j+2o+1]
                #   sub (x4,x3): out 8o+4-j     = ps0[512j+2o]   - s1[512j+2o+1]
                #   add (x1,x6): out 8o+1+5j    = ps0[512j+2o+1] + s1[512j+2o]
                #   sub (x5,x2): out 8o+5-3j    = ps0[512j+2o+1] - s1[512j+2o]
                def _ap3(base, off, jstep, ostep, ocnt):
                    a = base.copy()
                    part = a.ap.to_list()[0]
                    v = a.ap
                    v.clear()
                    v.extend([tuple(part), (jstep, 2), (ostep, ocnt)])
                    a.offset = a.offset + off
                    return a
                add, sub = nc.vector.tensor_add, nc.vector.tensor_sub
                if not last:
                    chunks = [(ps0, 0, HK, 0, 256, nc.sync)]
                else:
                    chunks = [(ps0a, 0, HK, 0, 128, nc.sync),
                              (ps0b, 0, HK, 256, 128, nc.scalar)]
                for pst, po, pjstep, so1, ocnt, dma_eng in chunks:
                    so = so1 * 4              # stage column offset of chunk
                    add(_ap3(stage[:], so + 0, 7, 8, ocnt),
                        _ap3(pst[:], po + 0, pjstep, 2, ocnt),
                        _ap3(s1[:], so1 + 1, HK, 2, ocnt))
                    sub(_ap3(stage[:], so + 4, -1, 8, ocnt),
                        _ap3(pst[:], po + 0, pjstep, 2, ocnt),
                        _ap3(s1[:], so1 + 1, HK, 2, ocnt))
                    add(_ap3(stage[:], so + 1, 5, 8, ocnt),
                        _ap3(pst[:], po + 1, pjstep, 2, ocnt),
                        _ap3(s1[:], so1 + 0, HK, 2, ocnt))
                    sub(_ap3(stage[:], so + 5, -3, 8, ocnt),
                        _ap3(pst[:], po + 1, pjstep, 2, ocnt),
                        _ap3(s1[:], so1 + 0, HK, 2, ocnt))
                    if last and so1 == 256:
                        # tail-critical store: two queues in parallel
                        half = ocnt * 4
                        nc.scalar.dma_start(
                            out[bt * 128:(bt + 1) * 128, so:so + half],
                            stage[:, so:so + half])
                        nc.sync.dma_start(
                            out[bt * 128:(bt + 1) * 128, so + half:so + ocnt * 8],
                            stage[:, so + half:so + ocnt * 8])
                    else:
                        dma_eng.dma_start(
                            out[bt * 128:(bt + 1) * 128, so:so + ocnt * 8],
                            stage[:, so:so + ocnt * 8])
    nc.finalize()
    return nc


def _pauli_parts(v):
    """v[..., 8] -> c0, c1 of shape [..., 2(m), 2(reim)]: the c-th column
    (Re, Im) of phi(v) rows m.  phi entries: A=P00=(v0+v4)+i(v3+v7),
    B=P01=(v1-v5)+i(v6-v2), C=P10=(v1+v5)+i(v6+v2), D=P11=(v0-v4)+i(v7-v3)."""
    c0 = np.empty(v.shape[:-1] + (2, 2), dtype=v.dtype)
    c1 = np.empty_like(c0)
    v0, v1, v2, v3, v4, v5, v6, v7 = (v[..., a] for a in range(8))
    c0[..., 0, 0] = v0 + v4   # Re A
    c0[..., 0, 1] = v3 + v7   # Im A
    c0[..., 1, 0] = v1 + v5   # Re C
    c0[..., 1, 1] = v6 + v2   # Im C
    c1[..., 0, 0] = v1 - v5   # Re B
    c1[..., 0, 1] = v6 - v2   # Im B
    c1[..., 1, 0] = v0 - v4   # Re D
    c1[..., 1, 1] = v7 - v3   # Im D
    return c0, c1


def _prep_w(weight):
    """weight [COUT, CIN, 8] -> [2, 512, 512] stacked R|I planes of
    phi(W)[r,m] indexed [(i,m), (o,r)], with the 0.5 inverse factor folded."""
    w = weight.astype(np.float32)
    # _pauli_parts returns matrix COLUMNS: cw_m[o,i,r,:] = (Re, Im) of
    # phi(W[o,i])[r, m].
    cw0, cw1 = _pauli_parts(w)
    R = np.empty((CIN, 2, COUT, 2), np.float32)   # [(i,m),(o,r)]
    I = np.empty_like(R)
    for m, cm in ((0, cw0), (1, cw1)):
        for r in range(2):
            R[:, m, :, r] = 0.5 * cm[:, :, r, 0].T
            I[:, m, :, r] = 0.5 * cm[:, :, r, 1].T
    return np.ascontiguousarray(
        np.stack([R.reshape(HK, HK), I.reshape(HK, HK)], axis=0))


def _prep_x(x):
    """x [B, CIN, 8] -> per-core xt arrays [N_CORES][BT, 128, KT*128] for
    c=0 and c=1, in the [bt, p, k, b] DMA-friendly layout.  Contraction row
    kappa = half*512 + i*2 + m  (half = 0:Re, 1:Im)."""
    xf = x.astype(np.float32)
    c0, c1 = _pauli_parts(xf)          # [B, CIN, m, reim]
    outs = []
    for arr in (c0, c1):
        # kappa-major array [K, B]: a = i*2+m ; kappa = ri*512 + a
        kb = arr.transpose(3, 1, 2, 0).reshape(K, B)   # [ri, i, m, b] -> [K, B]
        # device layout [core, bt, p, k, b]; kappa = k*128 + p
        a = kb.reshape(KT, 128, N_CORES, BT, 128)       # [k, p, core, bt, b]
        a = a.transpose(2, 3, 1, 0, 4)                  # [core, bt, p, k, b]
        outs.append(np.ascontiguousarray(
            a.reshape(N_CORES, BT, 128, KT * 128)))
    return outs


def kernel(x, weight, bias, cayley):
    assert x.shape == (B, CIN, NB) and weight.shape == (COUT, CIN, NB)
    if "nc" not in _cached:
        _cached["nc"] = _build_nc()
    nc = _cached["nc"]

    xt0, xt1 = _prep_x(np.asarray(x))
    wri = _prep_w(np.asarray(weight))
    in_maps = [{"xt0": xt0[c], "xt1": xt1[c], "wri": wri} for c in range(N_CORES)]
    res = run_bass_kernel_spmd(nc, in_maps, core_ids=list(range(N_CORES)))
    out = np.concatenate([res.results[c]["out"] for c in range(N_CORES)], axis=0)
    out = out.reshape(B, COUT, NB) + np.asarray(bias, np.float32)[None]
    return out.astype(np.float32)



# revision 28
# speedup vs baseline: 1.3112x; 1.3112x over previous
"""CliffordLinear (Cl(3,0)) Trainium2 kernel — Karatsuba complex matmul, fp16.

Math: Cl(3,0) ~ 2x2 complex matrices via Pauli rep phi.  The reference
out[b,o] = sum_i W[o,i] * X[b,i] (Clifford product) maps to
OutM[b,o] = sum_i phi(W[o,i]) @ phi(X[b,i]).  Per output column c of phi(X),
this is a complex matmul Out_c[b,(o,r)] = X_c[b,(i,m)] @ Wc[(i,m),(o,r)]
with Wc = R + iI shared by both c.  Karatsuba (3 real matmuls per complex
matmul instead of 4):

    M1 = XRe_c @ R;  M2 = XIm_c @ I;  M3 = (XRe_c + XIm_c) @ (R + I)
    OutRe_c = M1 - M2;   OutIm_c = M3 - M1 - M2

12.9G real MACs total (vs 17.2G for the 4-matmul split, 34.4G naive blade).
All matmul operands fp16 (PSUM accumulates f32): same PE rate as fp32r but
half the DMA traffic.  The Karatsuba sum operand ships pre-computed from the
host (DMA has slack; DVE does not).  PSUM results are evicted to fp16 SBUF
by the ACT engine; the Karatsuba combines and the inverse-Pauli butterfly
run on DVE in fp16 2x mode (all-SBUF, packed stride-1 last dims — weight
columns are laid out r-major and output blade-major to keep them packed).
Host undoes the blade-major layout with one transpose.

Sharding: data-parallel over batch (1024 rows/core); weights replicated.
Per-core HBM: 6 MB x + 1.5 MB w in, 4 MB out (~32 us DMA vs ~41 us PE).
"""

import sys

sys.path.insert(0, "/opt/trn_rl_repo")

import numpy as np

import concourse.bass as bass  # noqa: F401  (registers lowerings)
import concourse.mybir as mybir
import concourse.tile as tile
from concourse import bacc
from concourse.bass_utils import run_bass_kernel_spmd

N_CORES = 8
B, CIN, COUT, NB = 8192, 256, 256, 8
BS = B // N_CORES          # 1024 batch rows per core
HK = 512                   # contraction rows per Re/Im half: (i, m)
XW = 12 * 128              # x tile width: [Re k0-3 | Im k0-3 | Sum k0-3] x 128b
OUTW = COUT * NB           # 2048 output cols, BLADE-major: col = l*256 + o
BT = BS // 128             # 8 b-tiles

_cached = {}


def _ap3(base, off, dims):
    """Rewrite base's free AP to `dims` (list of (stride, count)), at
    element offset `off` into the tile row."""
    a = base.copy()
    part = a.ap.to_list()[0]
    v = a.ap
    v.clear()
    v.extend([tuple(part)] + [tuple(d) for d in dims])
    a.offset = a.offset + off
    return a


def _build_nc():
    f16 = mybir.dt.float16
    f32 = mybir.dt.float32
    nc = bacc.Bacc("TRN2", target_bir_lowering=False, debug=False,
                   num_devices=N_CORES)
    # x per column c: [bt, p, j, b]; row j*128+p: j 0-3 XRe k-tiles,
    # 4-7 XIm, 8-11 XSum (host-side XRe+XIm).
    xt0 = nc.dram_tensor("xt0", [BT, 128, XW], f16, kind="ExternalInput")
    xt1 = nc.dram_tensor("xt1", [BT, 128, XW], f16, kind="ExternalInput")
    # weight planes [R, I, R+I]: [(i,m), (r,o)] — columns r-major, rows
    # pre-tiled host-side to [p(128), k(4), col(512)] for a 1-trigger DMA.
    wri = nc.dram_tensor("wri", [3, 128, 4 * HK], f16, kind="ExternalInput")
    out = nc.dram_tensor("out", [BS, OUTW], f16, kind="ExternalOutput")

    with tile.TileContext(nc) as tc:
        with tc.tile_pool(name="wpool", bufs=1) as wpool, \
             tc.tile_pool(name="xpool", bufs=3) as xpool, \
             tc.tile_pool(name="epool", bufs=3) as epool, \
             tc.tile_pool(name="opool", bufs=3) as opool, \
             tc.tile_pool(name="pspool", bufs=1, space="PSUM") as pspool:
            # PE warmup: ramp the clock gate during the initial DMA wait,
            # sized to finish right as the first weight/x DMAs land.
            # Warmup accumulates into an m3_0-tagged PSUM buffer (no spare
            # bank: m12 tiles 2x2 banks + m3 tiles 2x2 banks = all 8).
            warm_in = wpool.tile([128, 640], mybir.dt.bfloat16, tag="warm_in")
            nc.vector.memset(warm_in[:], 0.0)
            warm_ps = pspool.tile([128, 512], f32, tag="m3_0", bufs=2,
                                  name="warm_ps")
            for _ in range(8):
                nc.tensor.matmul(warm_ps[:], warm_in[:, :128], warm_in[:, 128:640],
                                 start=True, stop=True)

            # Weight planes in SBUF: [128, k(4) x 512].
            wt = [wpool.tile([128, 4 * HK], f16, tag=f"w{p}", name=f"w{p}")
                  for p in range(3)]

            def load_w(p):
                nc.sync.dma_start(wt[p][:], wri[p])

            xt = [xt0, xt1]

            def load_x(bt, c, sum_now=True):
                xc = xpool.tile([128, XW], f16, tag=f"x{c}", name=f"x{c}")
                # Re+Im sections, then Sum section (separate DMAs so the
                # matmuls can start before the Sum lands).
                nc.sync.dma_start(xc[:, 0:1024], xt[c][bt][:, 0:1024])
                if sum_now:
                    load_x_sum(bt, c, xc)
                return xc

            def load_x_sum(bt, c, xc):
                nc.sync.dma_start(xc[:, 1024:1536], xt[c][bt][:, 1024:1536])

            # Startup order interleaves weight planes with bt0's x so the
            # PE never waits: R, x0[RI], I, x1[RI], R+I, x0[S], x1[S].
            load_w(0)
            x_cur = [None, None]
            x_cur[0] = load_x(0, 0, sum_now=False)
            load_w(1)
            x_cur[1] = load_x(0, 1, sum_now=False)
            load_w(2)
            load_x_sum(0, 0, x_cur[0])
            load_x_sum(0, 1, x_cur[1])

            m12 = [pspool.tile([128, 2 * HK], f32, tag=f"m12_{c}", name=f"m12_{c}")
                   for c in (0, 1)]
            m3s = {}

            def mm_group(c, g):
                """g: 0 = M1 (XRe@R), 1 = M2 (XIm@I), 2 = M3 (XSum@(R+I)).
                M3 gets its own double-buffered bank so the (late) M3 evict
                never gates the next b-tile's matmuls."""
                xc, w = x_cur[c], wt[g]
                if g == 2:
                    t = pspool.tile([128, HK], f32, tag=f"m3_{c}", bufs=2,
                                    name=f"m3_{c}")
                    m3s[c] = t
                    p = t[:]
                else:
                    p = m12[c][:, g * HK:(g + 1) * HK]
                for k in range(4):
                    nc.tensor.matmul(p,
                                     xc[:, (g * 4 + k) * 128:(g * 4 + k + 1) * 128],
                                     w[:, k * HK:(k + 1) * HK],
                                     start=(k == 0), stop=(k == 3))

            def combine12(c):
                """Evict M1|M2 to fp16 SBUF (ACT) and form D = M1-M2 (DVE).
                Issue right after (c,1) so ACT/DVE trail the PE closely."""
                e = epool.tile([128, 3 * HK], f16, tag=f"e{c}", name=f"e{c}")
                nc.scalar.copy(e[:, 0:1024], m12[c][:])
                d = epool.tile([128, 2 * HK], f16, tag=f"di{c}", name=f"di{c}")
                nc.vector.tensor_sub(d[:, 0:HK], e[:, 0:HK], e[:, HK:1024])
                return e, d

            def combine3(c, e, d):
                """Evict M3; I = (M3 - M1) - M2 into d's upper half."""
                nc.scalar.copy(e[:, 1024:1536], m3s[c][:])
                u = epool.tile([128, HK], f16, tag=f"u{c}", name=f"u{c}")
                nc.vector.tensor_sub(u[:], e[:, 1024:1536], e[:, 0:HK])
                nc.vector.tensor_sub(d[:, HK:1024], u[:], e[:, HK:1024])

            # Inverse Pauli butterfly, blade-major output.  DI layout
            # [D(r0)|D(r1)|I(r0)|I(r1)] x 256; j toggles Re/Im blade:
            #   x0,x7 = P[r0] + Q[r1]   x4,x3 = P[r0] - Q[r1]
            #   x1,x6 = P[r1] + Q[r0]   x5,x2 = P[r1] - Q[r0]
            O = 256
            BFLY = [(0, 7, 0, 1, True), (4, 3, 0, 1, False),
                    (1, 6, 1, 0, True), (5, 2, 1, 0, False)]

            def bfly(stage, P, Q, jsel=(0, 1)):
                """jsel=(0,1): dual-blade ops; (0,) or (1,): Re-only/Im-only
                single ops (tail pipelining)."""
                add, sub = nc.vector.tensor_add, nc.vector.tensor_sub
                for lre, lim, rp, rq, is_add in BFLY:
                    op = add if is_add else sub
                    if jsel == (0, 1):
                        op(_ap3(stage[:], lre * O, [((lim - lre) * O, 2), (1, O)]),
                           _ap3(P[:], rp * O, [(2 * O, 2), (1, O)]),
                           _ap3(Q[:], rq * O, [(2 * O, 2), (1, O)]))
                    else:
                        j = jsel[0]
                        l = lim if j else lre
                        op(_ap3(stage[:], l * O, [(1, O)]),
                           _ap3(P[:], (rp + 2 * j) * O, [(1, O)]),
                           _ap3(Q[:], (rq + 2 * j) * O, [(1, O)]))

            for bt in range(BT):
                last = bt == BT - 1
                if bt == 0:
                    # M3 groups last: their weights/x-sum arrive last.
                    order = [(0, 0), (0, 1), (1, 0), (1, 1), (0, 2), (1, 2)]
                elif last:
                    # c1's M3 last (issued split in r-halves below);
                    # everything c0 completes early so only c1's Im chain
                    # trails the final matmul.
                    order = [(1, 0), (1, 1), (0, 0), (0, 1), (0, 2)]
                else:
                    order = [(0, 0), (0, 1), (0, 2), (1, 0), (1, 1), (1, 2)]
                x_next = [None, None]
                if not last:
                    x_next[0] = load_x(bt + 1, 0)
                    x_next[1] = load_x(bt + 1, 1)
                ed = {}
                for c, g in order:
                    mm_group(c, g)
                    if g == 1:
                        ed[c] = combine12(c)
                    elif g == 2 and not (last and c == 1):
                        combine3(c, *ed[c])
                P, Q = ed[0][1], ed[1][1]
                stage = opool.tile([128, OUTW], f16, tag="stage")
                orow = out[bt * 128:(bt + 1) * 128]
                if not last:
                    bfly(stage, P, Q)
                    nc.gpsimd.dma_start(orow, stage[:])
                else:
                    # Tail: Re blades need only the D halves, which are ready
                    # before the final M3 group — issue them (and their
                    # stores) ahead of c1's combine3 so the in-order DVE
                    # queue doesn't park them behind the final evict.  Im
                    # blades + stores trail the last matmul, ordered so each
                    # store's blades complete first.
                    bfly(stage, P, Q, jsel=(0,))
                    nc.gpsimd.dma_start(orow[:, 0:2 * O], stage[:, 0:2 * O])
                    nc.gpsimd.dma_start(orow[:, 4 * O:6 * O], stage[:, 4 * O:6 * O])
                    combine3(1, *ed[1])
                    add, sub = nc.vector.tensor_add, nc.vector.tensor_sub
                    for lre, lim, rp, rq, is_add in (BFLY[1], BFLY[3]):  # x3, x2
                        (add if is_add else sub)(
                            _ap3(stage[:], lim * O, [(1, O)]),
                            _ap3(P[:], (rp + 2) * O, [(1, O)]),
                            _ap3(Q[:], (rq + 2) * O, [(1, O)]))
                    nc.scalar.dma_start(orow[:, 2 * O:4 * O], stage[:, 2 * O:4 * O])
                    for lre, lim, rp, rq, is_add in (BFLY[0], BFLY[2]):  # x7, x6
                        (add if is_add else sub)(
                            _ap3(stage[:], lim * O, [(1, O)]),
                            _ap3(P[:], (rp + 2) * O, [(1, O)]),
                            _ap3(Q[:], (rq + 2) * O, [(1, O)]))
                    nc.sync.dma_start(orow[:, 6 * O:8 * O], stage[:, 6 * O:8 * O])
                x_cur = x_next
    nc.finalize()
    return nc


def _pauli_parts(v):
    """v[..., 8] -> c0, c1 of shape [..., 2, 2]: the c-th column (rows, reim)
    of phi(v).  phi = [[A, B], [C, D]]: A=(v0+v4)+i(v3+v7), B=(v1-v5)+i(v6-v2),
    C=(v1+v5)+i(v6+v2), D=(v0-v4)+i(v7-v3)."""
    c0 = np.empty(v.shape[:-1] + (2, 2), dtype=v.dtype)
    c1 = np.empty_like(c0)
    v0, v1, v2, v3, v4, v5, v6, v7 = (v[..., a] for a in range(8))
    c0[..., 0, 0] = v0 + v4   # Re A
    c0[..., 0, 1] = v3 + v7   # Im A
    c0[..., 1, 0] = v1 + v5   # Re C
    c0[..., 1, 1] = v6 + v2   # Im C
    c1[..., 0, 0] = v1 - v5   # Re B
    c1[..., 0, 1] = v6 - v2   # Im B
    c1[..., 1, 0] = v0 - v4   # Re D
    c1[..., 1, 1] = v7 - v3   # Im D
    return c0, c1


def _prep_w(weight):
    """weight [COUT, CIN, 8] -> [3, 512, 512] fp16 planes R | I | R+I of
    phi(W)[r,m] indexed [(i,m), (r,o)] (columns r-major), 0.5 inverse-
    butterfly factor folded in."""
    w = weight.astype(np.float32)
    cw0, cw1 = _pauli_parts(w)     # cw_m[o,i,r,reim] = phi(W[o,i])[r,m]
    R = np.empty((CIN, 2, 2, COUT), np.float32)   # [i, m, r, o]
    I = np.empty_like(R)
    for m, cm in ((0, cw0), (1, cw1)):
        R[:, m] = 0.5 * cm[:, :, :, 0].transpose(1, 2, 0)
        I[:, m] = 0.5 * cm[:, :, :, 1].transpose(1, 2, 0)
    R = R.reshape(HK, HK)
    I = I.reshape(HK, HK)
    planes = np.stack([R, I, R + I])                  # [3, (i,m), (r,o)]
    # rows (i,m) -> [p(128), k(4)] tiling: plane[:, k*128+p, :] lands at
    # SBUF partition p, column block k.
    planes = planes.reshape(3, 4, 128, HK).transpose(0, 2, 1, 3)
    return np.ascontiguousarray(
        planes.reshape(3, 128, 4 * HK)).astype(np.float16)


def _prep_x(x):
    """x [B, CIN, 8] -> per-core fp16 arrays [N_CORES][BT, 128, 12*128] for
    c=0 and c=1: rows j*128+p = [XRe (i,m) | XIm (i,m) | XRe+XIm (i,m)]."""
    xf = x.astype(np.float32)
    c0, c1 = _pauli_parts(xf)          # [B, CIN, m, reim]
    outs = []
    for arr in (c0, c1):
        kb = arr.transpose(3, 1, 2, 0).reshape(2 * HK, B)  # [reim,i,m,b]
        full = np.concatenate([kb, kb[0:HK] + kb[HK:2 * HK]], axis=0)
        a = full.reshape(12, 128, N_CORES, BT, 128)        # [j, p, core, bt, b]
        a = a.transpose(2, 3, 1, 0, 4)                     # [core, bt, p, j, b]
        outs.append(np.ascontiguousarray(
            a.reshape(N_CORES, BT, 128, XW)).astype(np.float16))
    return outs


def kernel(x, weight, bias, cayley):
    assert x.shape == (B, CIN, NB) and weight.shape == (COUT, CIN, NB)
    if "nc" not in _cached:
        _cached["nc"] = _build_nc()
    nc = _cached["nc"]

    xt0, xt1 = _prep_x(np.asarray(x))
    wri = _prep_w(np.asarray(weight))
    in_maps = [{"xt0": xt0[c], "xt1": xt1[c], "wri": wri} for c in range(N_CORES)]
    res = run_bass_kernel_spmd(nc, in_maps, core_ids=list(range(N_CORES)))
    o = np.concatenate([res.results[c]["out"] for c in range(N_CORES)], axis=0)
    # blade-major [B, l, o] -> [B, o_channel, blade]
    o = o.reshape(B, NB, COUT).transpose(0, 2, 1).astype(np.float32)
    return o + np.asarray(bias, np.float32)[None]
